# revision 7
# baseline (speedup 1.0000x reference)
"""Trainium2 Bass kernel for nn_PatientMILFeatures (similarity-GCN + attention MIL pool).

Sharding: 4 bags x 2 cores/bag, row-split halves per bag. The odd core of each
pair gets half-rotated rows (the graph build / GCN / pooling are permutation
equivariant), so one SPMD program serves both pair members. Cross-core
exchanges use AllReduce(add) + subtract-recovery (no rank-dependent APs).

Numerics: similarity matmul in bf16 hi/lo (3 passes, ~1e-6 cos error) for
exact-enough top-8 selection via DVE max8 + match_replace (ties resolved to
lowest index, matching jax.lax.top_k). Adjacency stored bf16; GCN/attention
matmuls in f32r. Degree normalization folded into vector scalings.

Self-contained: hardcodes shapes; builds all constants on host.
"""
import sys
sys.path.insert(0, "/opt/trn_rl_repo")
import numpy as np
import ml_dtypes

import concourse.bacc as bacc
import concourse.tile as tile
import concourse.mybir as mybir
from concourse.bass_utils import run_bass_kernel_spmd

F32 = mybir.dt.float32
F32R = mybir.dt.float32r
BF16 = mybir.dt.bfloat16
AF = mybir.ActivationFunctionType
ALU = mybir.AluOpType
AX = mybir.AxisListType

B, N, D = 4, 2048, 768
KC = D // 128          # 6 contraction chunks
NB = N // 128          # 16 row/col blocks per bag
DA = 128               # attention dim
LN_EPS = 1e-5
RG = [[0, 1], [2, 3], [4, 5], [6, 7]]

PAIR = True
DEBUG = False


def build(pair=PAIR, debug=DEBUG):
    NLB = 8 if pair else 16
    NLOC = NLB * 128
    ncores = 8 if pair else 4
    nc = bacc.Bacc("TRN2", target_bir_lowering=False, debug=False,
                   num_devices=ncores)

    io = {}
    io["feats"] = nc.dram_tensor("feats", [N, D], F32, kind="ExternalInput").ap()
    io["wgt"] = nc.dram_tensor("wgt", [2, D, D], F32R, kind="ExternalInput").ap()
    io["lng"] = nc.dram_tensor("lng", [2, 128, D], F32, kind="ExternalInput").ap()
    io["lnb"] = nc.dram_tensor("lnb", [2, 128, D], F32, kind="ExternalInput").ap()
    io["og"] = nc.dram_tensor("og", [128, D], F32, kind="ExternalInput").ap()
    io["ob"] = nc.dram_tensor("ob", [128, D], F32, kind="ExternalInput").ap()
    io["w1t"] = nc.dram_tensor("w1t", [D, DA], F32R, kind="ExternalInput").ap()
    io["b1c"] = nc.dram_tensor("b1c", [DA, 1], F32, kind="ExternalInput").ap()
    io["w2c"] = nc.dram_tensor("w2c", [DA, 1], F32R, kind="ExternalInput").ap()
    io["b2s"] = nc.dram_tensor("b2s", [1, 1], F32, kind="ExternalInput").ap()
    io["wcr"] = nc.dram_tensor("wcr", [1, D], F32, kind="ExternalInput").ap()
    io["bcs"] = nc.dram_tensor("bcs", [1, 1], F32, kind="ExternalInput").ap()
    io["idf"] = nc.dram_tensor("idf", [128, 128], F32, kind="ExternalInput").ap()
    io["idb"] = nc.dram_tensor("idb", [128, 128], BF16, kind="ExternalInput").ap()
    io["aeye"] = nc.dram_tensor("aeye", [128, 128], F32, kind="ExternalInput").ap()
    io["onesc"] = nc.dram_tensor("onesc", [128, 1], F32R, kind="ExternalInput").ap()

    io["o_attn"] = nc.dram_tensor("o_attn", [1, NLOC], F32, kind="ExternalOutput").ap()
    io["o_pool"] = nc.dram_tensor("o_pool", [1, D], F32, kind="ExternalOutput").ap()
    io["o_log"] = nc.dram_tensor("o_log", [1, 1], F32, kind="ExternalOutput").ap()
    dbg = {}
    if debug:
        dbg["sim0"] = nc.dram_tensor("d_sim0", [128, N], F32, kind="ExternalOutput").ap()
        dbg["v80"] = nc.dram_tensor("d_v80", [128, 8], F32, kind="ExternalOutput").ap()
        dbg["T0"] = nc.dram_tensor("d_T0", [128, N], BF16, kind="ExternalOutput").ap()
        dbg["dis"] = nc.dram_tensor("d_dis", [128, NLB], F32, kind="ExternalOutput").ap()
        dbg["u0"] = nc.dram_tensor("d_u0", [128, D], BF16, kind="ExternalOutput").ap()
        dbg["x0"] = nc.dram_tensor("d_x0", [128, D], F32, kind="ExternalOutput").ap()
        dbg["xt0"] = nc.dram_tensor("d_xt0", [128, D], F32, kind="ExternalOutput").ap()
        dbg["lr"] = nc.dram_tensor("d_lr", [1, NLOC], F32, kind="ExternalOutput").ap()

    with tile.TileContext(nc) as tc:
        _body(nc, tc, pair, NLB, debug, dbg, io)
    nc.compile()
    return nc


def _body(nc, tc, pair, NLB, debug, dbg, io):
    NLOC = NLB * 128

    with tc.tile_pool(name="persist", bufs=1) as P, \
         tc.tile_pool(name="scratch", bufs=8) as SC, \
         tc.tile_pool(name="big", bufs=3) as BG, \
         tc.tile_pool(name="small", bufs=4) as SM, \
         tc.tile_pool(name="psum", bufs=3, space="PSUM") as PSF, \
         tc.tile_pool(name="psumb", bufs=2, space="PSUM") as PSB, \
         tc.tile_pool(name="dram", bufs=1, space="DRAM") as DR:

        def d768(dt=F32):
            return SC.tile([128, D], dt, tag="d768", name="d768")

        def d2048(rows=N, dt=F32):
            return BG.tile([128, rows], dt, tag="d2048", name="d2048")

        def pf(shape):
            return PSF.tile(shape, F32, tag="pf", name="pf")

        def pb():
            return PSB.tile([128, 1024], BF16, tag="pb", name="pb")

        # ---- persistent tiles (allocated up front; bottom of SBUF stack) ----
        idf = P.tile([128, 128], F32, tag="idf")
        idb = P.tile([128, 128], BF16, tag="idb")
        aeye = P.tile([128, 128], F32, tag="aeye")
        onesc = P.tile([128, 1], F32R, tag="onesc")
        lng = P.tile([128, 2, D], F32, tag="lng")
        lnb = P.tile([128, 2, D], F32, tag="lnb")
        og = P.tile([128, D], F32, tag="og")
        ob = P.tile([128, D], F32, tag="ob")
        w1t = P.tile([128, KC, DA], F32R, tag="w1t")
        b1c = P.tile([DA, 1], F32, tag="b1c")
        w2c = P.tile([DA, 1], F32R, tag="w2c")
        b2s = P.tile([1, 1], F32, tag="b2s")
        wcr = P.tile([1, D], F32, tag="wcr")
        bcs = P.tile([1, 1], F32, tag="bcs")
        x_my = P.tile([128, NLB, D], F32, tag="x_my")
        adjm = P.tile([128, NLB, N], BF16, tag="adjm")
        r_all = P.tile([128, NB], F32, tag="r_all")
        dis = P.tile([128, NLB], F32, tag="dis")
        wt = P.tile([128, KC, D], F32R, tag="wt")

        for name, tl in (("idf", idf), ("idb", idb), ("aeye", aeye),
                         ("onesc", onesc), ("lng", lng), ("lnb", lnb),
                         ("og", og), ("ob", ob), ("b1c", b1c), ("w2c", w2c),
                         ("b2s", b2s), ("wcr", wcr), ("bcs", bcs)):
            nc.sync.dma_start(tl[:], io[name][:])
        for k in range(KC):
            nc.sync.dma_start(w1t[:, k, :], io["w1t"][k * 128:(k + 1) * 128, :])
        feats = io["feats"]
        for i in range(NLB):
            nc.sync.dma_start(x_my[:, i, :], feats[i * 128:(i + 1) * 128, :])

        # ================= phase A: normalize + fnT hi/lo =================
        with tc.tile_pool(name="pFnT", bufs=1) as PF:
            fnT_hi = PF.tile([128, KC, N], BF16, tag="fnT_hi")
            fnT_lo = PF.tile([128, KC, N], BF16, tag="fnT_lo")
            for i in range(NB):
                if i < NLB:
                    xb = x_my[:, i, :]
                else:
                    xo = d768()
                    nc.sync.dma_start(
                        xo[:], feats[i * 128:(i + 1) * 128, :])
                    xb = xo[:]
                sq = d768()
                ss = SM.tile([128, 1], F32, tag="ss")
                nc.scalar.activation(sq[:], xb, AF.Square, accum_out=ss[:])
                nr = SM.tile([128, 1], F32, tag="nr")
                nc.scalar.activation(nr[:], ss[:], AF.Sqrt)
                nc.vector.tensor_scalar_max(nr[:], nr[:], 1e-8)
                nc.vector.reciprocal(r_all[:, i:i + 1], nr[:])
                fn = d768()
                nc.vector.tensor_scalar_mul(fn[:], xb, r_all[:, i:i + 1])
                pxt = pf([128, D])
                for k in range(KC):
                    nc.tensor.transpose(pxt[:, k * 128:(k + 1) * 128],
                                        fn[:, k * 128:(k + 1) * 128], idf[:])
                pxr = pxt[:].rearrange("p (k n) -> p k n", k=KC)
                hi_sl = fnT_hi[:, :, i * 128:(i + 1) * 128]
                nc.scalar.activation(hi_sl, pxr, AF.Copy)
                nc.vector.tensor_tensor(out=fnT_lo[:, :, i * 128:(i + 1) * 128],
                                        in0=pxr, in1=hi_sl, op=ALU.subtract)

            # ================= sim + top-8 per local row block =================
            for I in range(NLB):
                sim_sb = d2048()
                for ch in range(2):
                    ps = pf([128, 1024])
                    for c2 in range(2):
                        ops = [(pa, pb_, k)
                               for pa, pb_ in ((fnT_hi, fnT_hi), (fnT_hi, fnT_lo),
                                               (fnT_lo, fnT_hi))
                               for k in range(KC)]
                        for ix, (pa, pb_, k) in enumerate(ops):
                            col = ch * 1024 + c2 * 512
                            nc.tensor.matmul(
                                ps[:, c2 * 512:(c2 + 1) * 512],
                                pa[:, k, I * 128:(I + 1) * 128],
                                pb_[:, k, col:col + 512],
                                start=(ix == 0), stop=(ix == len(ops) - 1))
                    nc.scalar.activation(sim_sb[:, ch * 1024:(ch + 1) * 1024],
                                         ps[:], AF.Copy, bias=0.5, scale=0.5)
                nc.vector.tensor_tensor(out=sim_sb[:, I * 128:(I + 1) * 128],
                                        in0=sim_sb[:, I * 128:(I + 1) * 128],
                                        in1=aeye[:], op=ALU.mult)
                if debug and I == 0:
                    nc.sync.dma_start(dbg["sim0"][:], sim_sb[:])
                v8 = SM.tile([128, 8], F32, tag="v8")
                nc.vector.max(v8[:], sim_sb[:])
                if debug and I == 0:
                    nc.sync.dma_start(dbg["v80"][:], v8[:])
                repl = d2048()
                nc.vector.match_replace(repl[:], v8[:], sim_sb[:], 0.0)
                nc.vector.tensor_tensor(out=adjm[:, I, :], in0=sim_sb[:],
                                        in1=repl[:], op=ALU.subtract)

        # ================= symmetrize (+ pair exchange) =================
        with tc.tile_pool(name="phX", bufs=1) as PX:
            if pair:
                S_me = PX.tile([128, NLB, 1024], F32, tag="S_me")
                snd_adj = DR.tile([NLB, 128, 1024], F32, tag="snd_adj")
                rcv_adj = DR.tile([NLB, 128, 1024], F32, tag="rcv_adj")
                for cb in range(NLB):
                    ptr = pb()
                    for rb in range(NLB):
                        nc.tensor.transpose(
                            ptr[:, rb * 128:(rb + 1) * 128],
                            adjm[:, rb, 1024 + cb * 128:1024 + (cb + 1) * 128],
                            idb[:])
                    nc.scalar.activation(S_me[:, cb, :], ptr[:], AF.Copy)
                    nc.sync.dma_start(snd_adj[cb, :, :], S_me[:, cb, :])
                nc.gpsimd.collective_compute(
                    "AllReduce", ALU.add, replica_groups=RG,
                    ins=[snd_adj.opt()], outs=[rcv_adj.opt()])

            atr = PX.tile([128, NLB, NLOC], BF16, tag="atr")
            for cb in range(NLB):
                for hf in range(NLB // 8):
                    ptr = pb()
                    for r8 in range(8):
                        rb = hf * 8 + r8
                        nc.tensor.transpose(ptr[:, r8 * 128:(r8 + 1) * 128],
                                            adjm[:, rb, cb * 128:(cb + 1) * 128],
                                            idb[:])
                    nc.vector.tensor_copy(
                        atr[:, cb, hf * 1024:(hf + 1) * 1024], ptr[:])
            for I in range(NLB):
                nc.gpsimd.tensor_tensor(out=adjm[:, I, 0:NLOC],
                                        in0=adjm[:, I, 0:NLOC],
                                        in1=atr[:, I, :], op=ALU.add)
            if pair:
                for I in range(NLB):
                    t1 = d2048(1024)
                    nc.sync.dma_start(t1[:], rcv_adj[I, :, :])
                    part = d2048(1024)
                    nc.vector.tensor_tensor(out=part[:], in0=t1[:],
                                            in1=S_me[:, I, :], op=ALU.subtract)
                    nc.gpsimd.tensor_tensor(out=adjm[:, I, 1024:2048],
                                            in0=adjm[:, I, 1024:2048],
                                            in1=part[:], op=ALU.add)
        if debug:
            nc.sync.dma_start(dbg["T0"][:], adjm[:, 0, :])

        # deg -> dis = (0.5*deg + 1)^-1/2
        for I in range(NLB):
            dg = SM.tile([128, 1], F32, tag="dg")
            nc.vector.tensor_reduce(out=dg[:], in_=adjm[:, I, :], axis=AX.X,
                                    op=ALU.add)
            nc.vector.tensor_scalar(out=dg[:], in0=dg[:], scalar1=0.5,
                                    scalar2=1.0, op0=ALU.mult, op1=ALU.add)
            sr = SM.tile([128, 1], F32, tag="sr")
            nc.scalar.activation(sr[:], dg[:], AF.Sqrt)
            nc.vector.reciprocal(dis[:, I:I + 1], sr[:])
        if debug:
            nc.sync.dma_start(dbg["dis"][:], dis[:])

        # ================= GCN layers =================
        with tc.tile_pool(name="phG", bufs=1) as PG:
            if pair:
                adjT = PG.tile([128, NLB, NLOC], BF16, tag="adjT")
                for mc in range(NLB):
                    ptr = pb()
                    for nb_ in range(NLB):
                        nc.tensor.transpose(
                            ptr[:, nb_ * 128:(nb_ + 1) * 128],
                            adjm[:, nb_, 1024 + mc * 128:1024 + (mc + 1) * 128],
                            idb[:])
                    nc.vector.tensor_copy(adjT[:, mc, :], ptr[:])
                snd_u = DR.tile([NLB, 128, D], F32, tag="snd_u")
                rcv_u = DR.tile([NLB, 128, D], F32, tag="rcv_u")
            nmc = NB if pair else NLB
            u_all = PG.tile([128, nmc, D], BF16, tag="u_all")

            def lhsT_adj(mc, j):
                if mc < NLB:
                    return adjm[:, mc, j * 128:(j + 1) * 128]
                return adjT[:, mc - NLB, j * 128:(j + 1) * 128]

            for layer in range(2):
                for k in range(KC):
                    nc.sync.dma_start(
                        wt[:, k, :], io["wgt"][layer, k * 128:(k + 1) * 128, :])
                for i in range(NLB):
                    pxt = pf([128, D])
                    for k in range(KC):
                        nc.tensor.transpose(pxt[:, k * 128:(k + 1) * 128],
                                            x_my[:, i, k * 128:(k + 1) * 128],
                                            idf[:])
                    xtm = d768(F32R)
                    nc.scalar.activation(xtm[:], pxt[:], AF.Copy)
                    py = pf([128, D])
                    for c0, cw in ((0, 512), (512, 256)):
                        for k in range(KC):
                            nc.tensor.matmul(py[:, c0:c0 + cw],
                                             xtm[:, k * 128:(k + 1) * 128],
                                             wt[:, k, c0:c0 + cw],
                                             start=(k == 0), stop=(k == KC - 1))
                    nc.vector.tensor_scalar_mul(u_all[:, i, :], py[:],
                                                dis[:, i:i + 1])
                    if pair:
                        uf = d768()
                        nc.scalar.activation(uf[:], py[:], AF.Copy,
                                             scale=dis[:, i:i + 1])
                        nc.sync.dma_start(snd_u[i, :, :], uf[:])
                if pair:
                    nc.gpsimd.collective_compute(
                        "AllReduce", ALU.add, replica_groups=RG,
                        ins=[snd_u.opt()], outs=[rcv_u.opt()])
                    for i in range(NLB):
                        t1 = d768()
                        t2 = d768()
                        nc.sync.dma_start(t1[:], rcv_u[i, :, :])
                        nc.sync.dma_start(t2[:], snd_u[i, :, :])
                        nc.vector.tensor_tensor(out=u_all[:, NLB + i, :],
                                                in0=t1[:], in1=t2[:],
                                                op=ALU.subtract)
                if debug and layer == 0:
                    nc.sync.dma_start(dbg["u0"][:], u_all[:, 0, :])
                for j in range(NLB):
                    pz = pf([128, D])
                    for c0, cw in ((0, 512), (512, 256)):
                        for mc in range(nmc):
                            nc.tensor.matmul(pz[:, c0:c0 + cw],
                                             lhsT_adj(mc, j),
                                             u_all[:, mc, c0:c0 + cw],
                                             start=(mc == 0), stop=(mc == nmc - 1))
                    s1 = d768()
                    nc.vector.scalar_tensor_tensor(
                        out=s1[:], in0=pz[:], scalar=0.5, in1=u_all[:, j, :],
                        op0=ALU.mult, op1=ALU.add)
                    s1b = d768()
                    nc.gpsimd.tensor_scalar_mul(s1b[:], s1[:], dis[:, j:j + 1])
                    s2 = d768()
                    nc.gpsimd.tensor_tensor(out=s2[:], in0=s1b[:],
                                            in1=x_my[:, j, :], op=ALU.add)
                    _layernorm(nc, d768, SM, s2, lng[:, layer, :],
                               lnb[:, layer, :], x_my[:, j, :], relu=True)
                if debug and layer == 0:
                    nc.sync.dma_start(dbg["x0"][:], x_my[:, 0, :])

        # ================= output LN =================
        for j in range(NLB):
            s2 = d768()
            nc.vector.tensor_copy(s2[:], x_my[:, j, :])
            _layernorm(nc, d768, SM, s2, og[:], ob[:], x_my[:, j, :], relu=False)
        if debug:
            nc.sync.dma_start(dbg["xt0"][:], x_my[:, 0, :])

        # ================= attention pool =================
        with tc.tile_pool(name="phP", bufs=1) as PP:
            xtT = PP.tile([128, KC, NLOC], F32R, tag="xtT")
            for j in range(NLB):
                pxt = pf([128, D])
                for k in range(KC):
                    nc.tensor.transpose(pxt[:, k * 128:(k + 1) * 128],
                                        x_my[:, j, k * 128:(k + 1) * 128], idf[:])
                nc.scalar.activation(xtT[:, :, j * 128:(j + 1) * 128],
                                     pxt[:].rearrange("p (k n) -> p k n", k=KC),
                                     AF.Copy)
            ph = pf([128, 1024])
            for c2 in range(NLOC // 512):
                for k in range(KC):
                    nc.tensor.matmul(ph[:, c2 * 512:(c2 + 1) * 512], w1t[:, k, :],
                                     xtT[:, k, c2 * 512:(c2 + 1) * 512],
                                     start=(k == 0), stop=(k == KC - 1))
            hT = PP.tile([128, NLOC], F32R, tag="hT")
            nc.scalar.activation(hT[:], ph[:], AF.Tanh, bias=b1c[:])

            plr = pf([1, 1024])
            for c2 in range(NLOC // 512):
                nc.tensor.matmul(plr[:, c2 * 512:(c2 + 1) * 512], w2c[:],
                                 hT[:, c2 * 512:(c2 + 1) * 512],
                                 start=True, stop=True)
            attn_unr = PP.tile([1, NLOC], F32, tag="attn_unr")
            S_my = SM.tile([1, 1], F32, tag="S_my")
            nc.scalar.activation(attn_unr[:], plr[:, 0:NLOC], AF.Exp,
                                 bias=b2s[:], accum_out=S_my[:])
            if debug:
                nc.sync.dma_start(dbg["lr"][:], attn_unr[:])

            plc = pf([128, NLB])
            for j in range(NLB):
                nc.tensor.matmul(plc[:, j:j + 1],
                                 hT[:, j * 128:(j + 1) * 128].bitcast(F32),
                                 w2c[:].bitcast(F32), start=True, stop=True)
            b2bc = SM.tile([128, 1], F32, tag="b2bc")
            nc.gpsimd.partition_broadcast(b2bc[:], b2s[:])
            wcol = SM.tile([128, NLB], F32, tag="wcol")
            nc.scalar.activation(wcol[:], plc[:], AF.Exp, bias=b2bc[:])

            pP = pf([1, D])
            for j in range(NLB):
                sc = d768(F32R)
                nc.vector.tensor_scalar_mul(sc[:], x_my[:, j, :],
                                            wcol[:, j:j + 1])
                for c0, cw in ((0, 512), (512, 256)):
                    nc.tensor.matmul(pP[:, c0:c0 + cw], onesc[:],
                                     sc[:, c0:c0 + cw],
                                     start=(j == 0), stop=(j == NLB - 1))

            stat = PP.tile([1, D + 2], F32, tag="stat")
            nc.vector.tensor_copy(stat[:, 0:1], S_my[:])
            nc.vector.memset(stat[:, 1:2], 0.0)
            nc.scalar.activation(stat[:, 2:D + 2], pP[:], AF.Copy)
            if pair:
                snd_st = DR.tile([1, D + 2], F32, tag="snd_st")
                rcv_st = DR.tile([1, D + 2], F32, tag="rcv_st")
                nc.sync.dma_start(snd_st[:], stat[:])
                nc.gpsimd.collective_compute(
                    "AllReduce", ALU.add, replica_groups=RG,
                    ins=[snd_st.opt()], outs=[rcv_st.opt()])
                gst = PP.tile([1, D + 2], F32, tag="gst")
                nc.sync.dma_start(gst[:], rcv_st[:])
            else:
                gst = stat

            rS = SM.tile([1, 1], F32, tag="rS")
            nc.vector.reciprocal(rS[:], gst[:, 0:1])
            pooled = PP.tile([1, D], F32, tag="pooled")
            nc.vector.tensor_scalar_mul(pooled[:], gst[:, 2:D + 2], rS[:])
            nc.sync.dma_start(io["o_pool"][:], pooled[:])
            ao = PP.tile([1, NLOC], F32, tag="ao")
            nc.vector.tensor_scalar_mul(ao[:], attn_unr[:], rS[:])
            nc.sync.dma_start(io["o_attn"][:], ao[:])
            t5 = d768()
            nc.vector.tensor_tensor(out=t5[0:1, :], in0=pooled[:], in1=wcr[:],
                                    op=ALU.mult)
            t6 = SM.tile([1, 1], F32, tag="t6")
            nc.vector.tensor_reduce(out=t6[:], in_=t5[0:1, :], axis=AX.X,
                                    op=ALU.add)
            nc.vector.tensor_tensor(out=t6[:], in0=t6[:], in1=bcs[:], op=ALU.add)
            nc.sync.dma_start(io["o_log"][:], t6[:])


def _layernorm(nc, d768, SM, s2, g_row, b_row, out_ap, relu):
    """LN over free dim of s2 [128, D]; writes gamma/beta(+relu) to out_ap."""
    stats = SM.tile([128, 2, 6], F32, tag="lnstats")
    for c in range(2):
        nc.vector.bn_stats(stats[:, c, :], s2[:, c * 384:(c + 1) * 384])
    mv = SM.tile([128, 2], F32, tag="lnmv")
    nc.vector.bn_aggr(mv[:], stats[:])
    t3 = SM.tile([128, 1], F32, tag="lnt3")
    nc.vector.tensor_scalar_add(t3[:], mv[:, 1:2], LN_EPS)
    t4 = SM.tile([128, 1], F32, tag="lnt4")
    nc.scalar.activation(t4[:], t3[:], AF.Sqrt)
    rstd = SM.tile([128, 1], F32, tag="lnrstd")
    nc.vector.reciprocal(rstd[:], t4[:])
    s3 = d768()
    nc.vector.tensor_scalar(out=s3[:], in0=s2[:], scalar1=mv[:, 0:1],
                            scalar2=rstd[:], op0=ALU.subtract, op1=ALU.mult)
    s4 = d768()
    nc.gpsimd.tensor_tensor(out=s4[:], in0=s3[:], in1=g_row, op=ALU.mult)
    if relu:
        s5 = d768()
        nc.gpsimd.tensor_tensor(out=s5[:], in0=s4[:], in1=b_row, op=ALU.add)
        nc.scalar.activation(out_ap, s5[:], AF.Relu)
    else:
        nc.gpsimd.tensor_tensor(out=out_ap, in0=s4[:], in1=b_row, op=ALU.add)


_CACHE = {}


def _get_prog(pair=PAIR, debug=DEBUG):
    key = (pair, debug)
    if key not in _CACHE:
        _CACHE[key] = build(pair, debug)
    return _CACHE[key]


def make_in_maps(feats, W_gcn, ln_g, ln_b, outln_g, outln_b, W1, b1, W2, b2,
                 Wc, bc, pair=PAIR):
    f32 = np.float32
    feats = np.asarray(feats, f32)
    common = {
        "wgt": np.ascontiguousarray(np.asarray(W_gcn, f32).transpose(0, 2, 1)),
        "lng": np.ascontiguousarray(np.broadcast_to(
            np.asarray(ln_g, f32)[:, None, :], (2, 128, D))),
        "lnb": np.ascontiguousarray(np.broadcast_to(
            np.asarray(ln_b, f32)[:, None, :], (2, 128, D))),
        "og": np.ascontiguousarray(np.broadcast_to(
            np.asarray(outln_g, f32)[None, :], (128, D))),
        "ob": np.ascontiguousarray(np.broadcast_to(
            np.asarray(outln_b, f32)[None, :], (128, D))),
        "w1t": np.ascontiguousarray(np.asarray(W1, f32).T),
        "b1c": np.asarray(b1, f32).reshape(DA, 1),
        "w2c": np.ascontiguousarray(np.asarray(W2, f32).reshape(1, DA).T),
        "b2s": np.asarray(b2, f32).reshape(1, 1),
        "wcr": np.asarray(Wc, f32).reshape(1, D),
        "bcs": np.asarray(bc, f32).reshape(1, 1),
        "idf": np.eye(128, dtype=f32),
        "idb": np.eye(128, dtype=ml_dtypes.bfloat16),
        "aeye": (1.0 - np.eye(128)).astype(f32),
        "onesc": np.ones((128, 1), f32),
    }
    in_maps = []
    ncores = 8 if pair else 4
    for c in range(ncores):
        bag = c // 2 if pair else c
        h = c % 2 if pair else 0
        fb = feats[bag]
        if h == 1:
            fb = np.concatenate([fb[1024:], fb[:1024]], axis=0)
        d = dict(common)
        d["feats"] = np.ascontiguousarray(fb)
        in_maps.append(d)
    return in_maps


def run(inputs, pair=PAIR, debug=DEBUG, **spmd_kwargs):
    nc = _get_prog(pair, debug)
    in_maps = make_in_maps(
        inputs["feats"], inputs["W_gcn"], inputs["ln_g"], inputs["ln_b"],
        inputs["outln_g"], inputs["outln_b"], inputs["W1"], inputs["b1"],
        inputs["W2"], inputs["b2"], inputs["Wc"], inputs["bc"], pair=pair)
    ncores = 8 if pair else 4
    res = run_bass_kernel_spmd(nc, in_maps, core_ids=list(range(ncores)),
                               **spmd_kwargs)
    rs = res.results
    logits = np.zeros((B, 1), np.float32)
    pooled = np.zeros((B, D), np.float32)
    attn = np.zeros((B, N), np.float32)
    for bag in range(B):
        if pair:
            ev, od = rs[2 * bag], rs[2 * bag + 1]
            logits[bag, 0] = ev["o_log"][0, 0]
            pooled[bag] = ev["o_pool"][0]
            attn[bag, :1024] = ev["o_attn"][0]
            attn[bag, 1024:] = od["o_attn"][0]
        else:
            r = rs[bag]
            logits[bag, 0] = r["o_log"][0, 0]
            pooled[bag] = r["o_pool"][0]
            attn[bag] = r["o_attn"][0]
    return (logits, pooled, attn), res


def kernel(**inputs):
    out, _ = run(inputs)
    return out


if __name__ == "__main__":
    import reference
    ins = {k: np.asarray(v) for k, v in reference.setup_inputs().items()}
    out = kernel(**ins)
    print("logits:", out[0].ravel())


# revision 9
# speedup vs baseline: 3536.4518x; 3536.4518x over previous
"""Trainium2 Bass kernel for nn_PatientMILFeatures (similarity-GCN + attention MIL pool).

Sharding: 4 bags x 2 cores/bag, row-split halves per bag. The odd core of each
pair gets half-rotated rows (the graph build / GCN / pooling are permutation
equivariant), so one SPMD program serves both pair members. Cross-core
exchanges use AllReduce(add) + subtract-recovery (no rank-dependent APs).

Numerics: similarity matmul in bf16 hi/lo (3 passes, ~1e-6 cos error) for
exact-enough top-8 selection via DVE max8 + match_replace (ties resolved to
lowest index, matching jax.lax.top_k). Adjacency stored bf16; GCN/attention
matmuls in f32r. Degree normalization folded into vector scalings.

Self-contained: hardcodes shapes; builds all constants on host.
"""
import sys
sys.path.insert(0, "/opt/trn_rl_repo")
import numpy as np
import ml_dtypes

import concourse.bacc as bacc
import concourse.tile as tile
import concourse.mybir as mybir
from concourse.bass_utils import run_bass_kernel_spmd

F32 = mybir.dt.float32
F32R = mybir.dt.float32r
BF16 = mybir.dt.bfloat16
AF = mybir.ActivationFunctionType
ALU = mybir.AluOpType
AX = mybir.AxisListType

B, N, D = 4, 2048, 768
KC = D // 128          # 6 contraction chunks
NB = N // 128          # 16 row/col blocks per bag
DA = 128               # attention dim
LN_EPS = 1e-5
RG = [[0, 1], [2, 3], [4, 5], [6, 7]]

PAIR = True
DEBUG = False


def build(pair=PAIR, debug=DEBUG):
    NLB = 8 if pair else 16
    NLOC = NLB * 128
    ncores = 8 if pair else 4
    nc = bacc.Bacc("TRN2", target_bir_lowering=False, debug=False,
                   num_devices=ncores)

    io = {}
    io["feats"] = nc.dram_tensor("feats", [N, D], F32, kind="ExternalInput").ap()
    io["wgt"] = nc.dram_tensor("wgt", [2, D, D], F32R, kind="ExternalInput").ap()
    io["lng"] = nc.dram_tensor("lng", [2, 128, D], F32, kind="ExternalInput").ap()
    io["lnb"] = nc.dram_tensor("lnb", [2, 128, D], F32, kind="ExternalInput").ap()
    io["og"] = nc.dram_tensor("og", [128, D], F32, kind="ExternalInput").ap()
    io["ob"] = nc.dram_tensor("ob", [128, D], F32, kind="ExternalInput").ap()
    io["w1t"] = nc.dram_tensor("w1t", [D, DA], F32R, kind="ExternalInput").ap()
    io["b1c"] = nc.dram_tensor("b1c", [DA, 1], F32, kind="ExternalInput").ap()
    io["w2c"] = nc.dram_tensor("w2c", [DA, 1], F32R, kind="ExternalInput").ap()
    io["b2s"] = nc.dram_tensor("b2s", [1, 1], F32, kind="ExternalInput").ap()
    io["wcr"] = nc.dram_tensor("wcr", [1, D], F32, kind="ExternalInput").ap()
    io["bcs"] = nc.dram_tensor("bcs", [1, 1], F32, kind="ExternalInput").ap()
    io["idf"] = nc.dram_tensor("idf", [128, 128], F32, kind="ExternalInput").ap()
    io["idb"] = nc.dram_tensor("idb", [128, 128], BF16, kind="ExternalInput").ap()
    io["aeye"] = nc.dram_tensor("aeye", [128, 128], F32, kind="ExternalInput").ap()
    io["onesc"] = nc.dram_tensor("onesc", [128, 1], F32R, kind="ExternalInput").ap()

    io["o_attn"] = nc.dram_tensor("o_attn", [1, NLOC], F32, kind="ExternalOutput").ap()
    io["o_pool"] = nc.dram_tensor("o_pool", [1, D], F32, kind="ExternalOutput").ap()
    io["o_log"] = nc.dram_tensor("o_log", [1, 1], F32, kind="ExternalOutput").ap()
    dbg = {}
    if debug:
        dbg["sim0"] = nc.dram_tensor("d_sim0", [128, N], F32, kind="ExternalOutput").ap()
        dbg["v80"] = nc.dram_tensor("d_v80", [128, 8], F32, kind="ExternalOutput").ap()
        dbg["T0"] = nc.dram_tensor("d_T0", [128, N], BF16, kind="ExternalOutput").ap()
        dbg["dis"] = nc.dram_tensor("d_dis", [128, NLB], F32, kind="ExternalOutput").ap()
        dbg["u0"] = nc.dram_tensor("d_u0", [128, D], BF16, kind="ExternalOutput").ap()
        dbg["x0"] = nc.dram_tensor("d_x0", [128, D], F32, kind="ExternalOutput").ap()
        dbg["xt0"] = nc.dram_tensor("d_xt0", [128, D], F32, kind="ExternalOutput").ap()
        dbg["lr"] = nc.dram_tensor("d_lr", [1, NLOC], F32, kind="ExternalOutput").ap()

    with tile.TileContext(nc) as tc:
        _body(nc, tc, pair, NLB, debug, dbg, io)
    nc.compile()
    return nc


def _body(nc, tc, pair, NLB, debug, dbg, io):
    NLOC = NLB * 128

    with tc.tile_pool(name="persist", bufs=1) as P, \
         tc.tile_pool(name="scratch", bufs=8) as SC, \
         tc.tile_pool(name="big", bufs=3) as BG, \
         tc.tile_pool(name="small", bufs=4) as SM, \
         tc.tile_pool(name="psum", bufs=3, space="PSUM") as PSF, \
         tc.tile_pool(name="psumb", bufs=2, space="PSUM") as PSB, \
         tc.tile_pool(name="dram", bufs=1, space="DRAM") as DR:

        def d768(dt=F32):
            return SC.tile([128, D], dt, tag="d768", name="d768")

        def d2048(rows=N, dt=F32):
            return BG.tile([128, rows], dt, tag="d2048", name="d2048")

        def pf(shape):
            return PSF.tile(shape, F32, tag="pf", name="pf")

        def pb():
            return PSB.tile([128, 1024], BF16, tag="pb", name="pb")

        # ---- persistent tiles (allocated up front; bottom of SBUF stack) ----
        idf = P.tile([128, 128], F32, tag="idf")
        idb = P.tile([128, 128], BF16, tag="idb")
        aeye = P.tile([128, 128], F32, tag="aeye")
        onesc = P.tile([128, 1], F32R, tag="onesc")
        lng = P.tile([128, 2, D], F32, tag="lng")
        lnb = P.tile([128, 2, D], F32, tag="lnb")
        og = P.tile([128, D], F32, tag="og")
        ob = P.tile([128, D], F32, tag="ob")
        w1t = P.tile([128, KC, DA], F32R, tag="w1t")
        b1c = P.tile([DA, 1], F32, tag="b1c")
        w2c = P.tile([DA, 1], F32R, tag="w2c")
        b2s = P.tile([1, 1], F32, tag="b2s")
        wcr = P.tile([1, D], F32, tag="wcr")
        bcs = P.tile([1, 1], F32, tag="bcs")
        x_my = P.tile([128, NLB, D], F32, tag="x_my")
        adjm = P.tile([128, NLB, N], BF16, tag="adjm")
        r_all = P.tile([128, NB], F32, tag="r_all")
        dis = P.tile([128, NLB], F32, tag="dis")
        wt = P.tile([128, KC, D], F32R, tag="wt")

        for name, tl in (("idf", idf), ("idb", idb), ("aeye", aeye),
                         ("onesc", onesc), ("lng", lng), ("lnb", lnb),
                         ("og", og), ("ob", ob), ("b1c", b1c), ("w2c", w2c),
                         ("b2s", b2s), ("wcr", wcr), ("bcs", bcs)):
            nc.sync.dma_start(tl[:], io[name][:])
        for k in range(KC):
            nc.sync.dma_start(w1t[:, k, :], io["w1t"][k * 128:(k + 1) * 128, :])
        feats = io["feats"]
        for i in range(NLB):
            nc.sync.dma_start(x_my[:, i, :], feats[i * 128:(i + 1) * 128, :])

        # ================= phase A: normalize + fnT hi/lo =================
        with tc.tile_pool(name="pFnT", bufs=1) as PF:
            fnT_hi = PF.tile([128, KC, N], BF16, tag="fnT_hi")
            fnT_lo = PF.tile([128, KC, N], BF16, tag="fnT_lo")
            for i in range(NB):
                if i < NLB:
                    xb = x_my[:, i, :]
                else:
                    xo = d768()
                    nc.sync.dma_start(
                        xo[:], feats[i * 128:(i + 1) * 128, :])
                    xb = xo[:]
                sq = d768()
                ss = SM.tile([128, 1], F32, tag="ss")
                nc.scalar.activation(sq[:], xb, AF.Square, accum_out=ss[:])
                nr = SM.tile([128, 1], F32, tag="nr")
                nc.scalar.activation(nr[:], ss[:], AF.Sqrt)
                nc.vector.tensor_scalar_max(nr[:], nr[:], 1e-8)
                nc.vector.reciprocal(r_all[:, i:i + 1], nr[:])
                fn = d768()
                nc.vector.tensor_scalar_mul(fn[:], xb, r_all[:, i:i + 1])
                pxt = pf([128, D])
                for k in range(KC):
                    nc.tensor.transpose(pxt[:, k * 128:(k + 1) * 128],
                                        fn[:, k * 128:(k + 1) * 128], idf[:])
                pxr = pxt[:].rearrange("p (k n) -> p k n", k=KC)
                hi_sl = fnT_hi[:, :, i * 128:(i + 1) * 128]
                nc.scalar.activation(hi_sl, pxr, AF.Copy)
                nc.vector.tensor_tensor(out=fnT_lo[:, :, i * 128:(i + 1) * 128],
                                        in0=pxr, in1=hi_sl, op=ALU.subtract)

            # ================= sim + top-8 per local row block =================
            for I in range(NLB):
                sim_sb = d2048()
                for ch in range(2):
                    ps = pf([128, 1024])
                    for c2 in range(2):
                        ops = [(pa, pb_, k)
                               for pa, pb_ in ((fnT_hi, fnT_hi), (fnT_hi, fnT_lo),
                                               (fnT_lo, fnT_hi))
                               for k in range(KC)]
                        for ix, (pa, pb_, k) in enumerate(ops):
                            col = ch * 1024 + c2 * 512
                            nc.tensor.matmul(
                                ps[:, c2 * 512:(c2 + 1) * 512],
                                pa[:, k, I * 128:(I + 1) * 128],
                                pb_[:, k, col:col + 512],
                                start=(ix == 0), stop=(ix == len(ops) - 1))
                    nc.scalar.activation(sim_sb[:, ch * 1024:(ch + 1) * 1024],
                                         ps[:], AF.Copy, bias=0.5, scale=0.5)
                nc.vector.tensor_tensor(out=sim_sb[:, I * 128:(I + 1) * 128],
                                        in0=sim_sb[:, I * 128:(I + 1) * 128],
                                        in1=aeye[:], op=ALU.mult)
                if debug and I == 0:
                    nc.sync.dma_start(dbg["sim0"][:], sim_sb[:])
                v8 = SM.tile([128, 8], F32, tag="v8")
                nc.vector.max(v8[:], sim_sb[:])
                if debug and I == 0:
                    nc.sync.dma_start(dbg["v80"][:], v8[:])
                repl = d2048()
                nc.vector.match_replace(repl[:], v8[:], sim_sb[:], 0.0)
                nc.vector.tensor_tensor(out=adjm[:, I, :], in0=sim_sb[:],
                                        in1=repl[:], op=ALU.subtract)

        # ================= symmetrize (+ pair exchange) =================
        with tc.tile_pool(name="phX", bufs=1) as PX:
            if pair:
                S_me = PX.tile([128, NLB, 1024], F32, tag="S_me")
                snd_adj = DR.tile([NLB, 128, 1024], F32, tag="snd_adj")
                rcv_adj = DR.tile([NLB, 128, 1024], F32, tag="rcv_adj")
                for cb in range(NLB):
                    ptr = pb()
                    for rb in range(NLB):
                        nc.tensor.transpose(
                            ptr[:, rb * 128:(rb + 1) * 128],
                            adjm[:, rb, 1024 + cb * 128:1024 + (cb + 1) * 128],
                            idb[:])
                    nc.scalar.activation(S_me[:, cb, :], ptr[:], AF.Copy)
                    nc.sync.dma_start(snd_adj[cb, :, :], S_me[:, cb, :])
                nc.gpsimd.collective_compute(
                    "AllReduce", ALU.add, replica_groups=RG,
                    ins=[snd_adj.opt()], outs=[rcv_adj.opt()])

            atr = PX.tile([128, NLB, NLOC], BF16, tag="atr")
            for cb in range(NLB):
                for hf in range(NLB // 8):
                    ptr = pb()
                    for r8 in range(8):
                        rb = hf * 8 + r8
                        nc.tensor.transpose(ptr[:, r8 * 128:(r8 + 1) * 128],
                                            adjm[:, rb, cb * 128:(cb + 1) * 128],
                                            idb[:])
                    nc.vector.tensor_copy(
                        atr[:, cb, hf * 1024:(hf + 1) * 1024], ptr[:])
            for I in range(NLB):
                nc.vector.tensor_tensor(out=adjm[:, I, 0:NLOC],
                                        in0=adjm[:, I, 0:NLOC],
                                        in1=atr[:, I, :], op=ALU.add)
            if pair:
                for I in range(NLB):
                    t1 = d2048(1024)
                    nc.sync.dma_start(t1[:], rcv_adj[I, :, :])
                    part = d2048(1024)
                    nc.vector.tensor_tensor(out=part[:], in0=t1[:],
                                            in1=S_me[:, I, :], op=ALU.subtract)
                    nc.vector.tensor_tensor(out=adjm[:, I, 1024:2048],
                                            in0=adjm[:, I, 1024:2048],
                                            in1=part[:], op=ALU.add)
        if debug:
            nc.sync.dma_start(dbg["T0"][:], adjm[:, 0, :])

        # deg -> dis = (0.5*deg + 1)^-1/2
        for I in range(NLB):
            dg = SM.tile([128, 1], F32, tag="dg")
            nc.vector.tensor_reduce(out=dg[:], in_=adjm[:, I, :], axis=AX.X,
                                    op=ALU.add)
            nc.vector.tensor_scalar(out=dg[:], in0=dg[:], scalar1=0.5,
                                    scalar2=1.0, op0=ALU.mult, op1=ALU.add)
            sr = SM.tile([128, 1], F32, tag="sr")
            nc.scalar.activation(sr[:], dg[:], AF.Sqrt)
            nc.vector.reciprocal(dis[:, I:I + 1], sr[:])
        if debug:
            nc.sync.dma_start(dbg["dis"][:], dis[:])

        # ================= GCN layers =================
        with tc.tile_pool(name="phG", bufs=1) as PG:
            if pair:
                adjT = PG.tile([128, NLB, NLOC], BF16, tag="adjT")
                for mc in range(NLB):
                    ptr = pb()
                    for nb_ in range(NLB):
                        nc.tensor.transpose(
                            ptr[:, nb_ * 128:(nb_ + 1) * 128],
                            adjm[:, nb_, 1024 + mc * 128:1024 + (mc + 1) * 128],
                            idb[:])
                    nc.vector.tensor_copy(adjT[:, mc, :], ptr[:])
                snd_u = DR.tile([NLB, 128, D], F32, tag="snd_u")
                rcv_u = DR.tile([NLB, 128, D], F32, tag="rcv_u")
            nmc = NB if pair else NLB
            u_all = PG.tile([128, nmc, D], BF16, tag="u_all")

            def lhsT_adj(mc, j):
                if mc < NLB:
                    return adjm[:, mc, j * 128:(j + 1) * 128]
                return adjT[:, mc - NLB, j * 128:(j + 1) * 128]

            for layer in range(2):
                for k in range(KC):
                    nc.sync.dma_start(
                        wt[:, k, :], io["wgt"][layer, k * 128:(k + 1) * 128, :])
                for i in range(NLB):
                    pxt = pf([128, D])
                    for k in range(KC):
                        nc.tensor.transpose(pxt[:, k * 128:(k + 1) * 128],
                                            x_my[:, i, k * 128:(k + 1) * 128],
                                            idf[:])
                    xtm = d768(F32R)
                    nc.scalar.activation(xtm[:], pxt[:], AF.Copy)
                    py = pf([128, D])
                    for k in range(KC):
                        for c0, cw in ((0, 512), (512, 256)):
                            nc.tensor.matmul(py[:, c0:c0 + cw],
                                             xtm[:, k * 128:(k + 1) * 128],
                                             wt[:, k, c0:c0 + cw],
                                             start=(k == 0), stop=(k == KC - 1))
                    nc.vector.tensor_scalar_mul(u_all[:, i, :], py[:],
                                                dis[:, i:i + 1])
                    if pair:
                        uf = d768()
                        nc.scalar.activation(uf[:], py[:], AF.Copy,
                                             scale=dis[:, i:i + 1])
                        nc.sync.dma_start(snd_u[i, :, :], uf[:])
                if pair:
                    nc.gpsimd.collective_compute(
                        "AllReduce", ALU.add, replica_groups=RG,
                        ins=[snd_u.opt()], outs=[rcv_u.opt()])
                    for i in range(NLB):
                        t1 = d768()
                        t2 = d768()
                        nc.sync.dma_start(t1[:], rcv_u[i, :, :])
                        nc.sync.dma_start(t2[:], snd_u[i, :, :])
                        nc.vector.tensor_tensor(out=u_all[:, NLB + i, :],
                                                in0=t1[:], in1=t2[:],
                                                op=ALU.subtract)
                if debug and layer == 0:
                    nc.sync.dma_start(dbg["u0"][:], u_all[:, 0, :])
                for j in range(NLB):
                    pz = pf([128, D])
                    for mc in range(nmc):
                        for c0, cw in ((0, 512), (512, 256)):
                            nc.tensor.matmul(pz[:, c0:c0 + cw],
                                             lhsT_adj(mc, j),
                                             u_all[:, mc, c0:c0 + cw],
                                             start=(mc == 0), stop=(mc == nmc - 1))
                    s1 = d768()
                    nc.vector.scalar_tensor_tensor(
                        out=s1[:], in0=pz[:], scalar=0.5, in1=u_all[:, j, :],
                        op0=ALU.mult, op1=ALU.add)
                    s1b = d768()
                    nc.gpsimd.tensor_scalar_mul(s1b[:], s1[:], dis[:, j:j + 1])
                    s2 = d768()
                    nc.gpsimd.tensor_tensor(out=s2[:], in0=s1b[:],
                                            in1=x_my[:, j, :], op=ALU.add)
                    _layernorm(nc, d768, SM, s2, lng[:, layer, :],
                               lnb[:, layer, :], x_my[:, j, :], relu=True)
                if debug and layer == 0:
                    nc.sync.dma_start(dbg["x0"][:], x_my[:, 0, :])

        # ================= output LN =================
        for j in range(NLB):
            s2 = d768()
            nc.vector.tensor_copy(s2[:], x_my[:, j, :])
            _layernorm(nc, d768, SM, s2, og[:], ob[:], x_my[:, j, :], relu=False)
        if debug:
            nc.sync.dma_start(dbg["xt0"][:], x_my[:, 0, :])

        # ================= attention pool =================
        with tc.tile_pool(name="phP", bufs=1) as PP:
            xtT = PP.tile([128, KC, NLOC], F32R, tag="xtT")
            for j in range(NLB):
                pxt = pf([128, D])
                for k in range(KC):
                    nc.tensor.transpose(pxt[:, k * 128:(k + 1) * 128],
                                        x_my[:, j, k * 128:(k + 1) * 128], idf[:])
                nc.scalar.activation(xtT[:, :, j * 128:(j + 1) * 128],
                                     pxt[:].rearrange("p (k n) -> p k n", k=KC),
                                     AF.Copy)
            ph = pf([128, 1024])
            for c2 in range(NLOC // 512):
                for k in range(KC):
                    nc.tensor.matmul(ph[:, c2 * 512:(c2 + 1) * 512], w1t[:, k, :],
                                     xtT[:, k, c2 * 512:(c2 + 1) * 512],
                                     start=(k == 0), stop=(k == KC - 1))
            hT = PP.tile([128, NLOC], F32R, tag="hT")
            nc.scalar.activation(hT[:], ph[:], AF.Tanh, bias=b1c[:])

            plr = pf([1, 1024])
            for c2 in range(NLOC // 512):
                nc.tensor.matmul(plr[:, c2 * 512:(c2 + 1) * 512], w2c[:],
                                 hT[:, c2 * 512:(c2 + 1) * 512],
                                 start=True, stop=True)
            attn_unr = PP.tile([1, NLOC], F32, tag="attn_unr")
            S_my = SM.tile([1, 1], F32, tag="S_my")
            nc.scalar.activation(attn_unr[:], plr[:, 0:NLOC], AF.Exp,
                                 bias=b2s[:], accum_out=S_my[:])
            if debug:
                nc.sync.dma_start(dbg["lr"][:], attn_unr[:])

            plc = pf([128, NLB])
            for j in range(NLB):
                nc.tensor.matmul(plc[:, j:j + 1],
                                 hT[:, j * 128:(j + 1) * 128].bitcast(F32),
                                 w2c[:].bitcast(F32), start=True, stop=True)
            b2bc = SM.tile([128, 1], F32, tag="b2bc")
            nc.gpsimd.partition_broadcast(b2bc[:], b2s[:])
            wcol = SM.tile([128, NLB], F32, tag="wcol")
            nc.scalar.activation(wcol[:], plc[:], AF.Exp, bias=b2bc[:])

            pP = pf([1, D])
            for j in range(NLB):
                sc = d768(F32R)
                nc.vector.tensor_scalar_mul(sc[:], x_my[:, j, :],
                                            wcol[:, j:j + 1])
                for c0, cw in ((0, 512), (512, 256)):
                    nc.tensor.matmul(pP[:, c0:c0 + cw], onesc[:],
                                     sc[:, c0:c0 + cw],
                                     start=(j == 0), stop=(j == NLB - 1))

            stat = PP.tile([1, D + 2], F32, tag="stat")
            nc.vector.tensor_copy(stat[:, 0:1], S_my[:])
            nc.vector.memset(stat[:, 1:2], 0.0)
            nc.scalar.activation(stat[:, 2:D + 2], pP[:], AF.Copy)
            if pair:
                snd_st = DR.tile([1, D + 2], F32, tag="snd_st")
                rcv_st = DR.tile([1, D + 2], F32, tag="rcv_st")
                nc.sync.dma_start(snd_st[:], stat[:])
                nc.gpsimd.collective_compute(
                    "AllReduce", ALU.add, replica_groups=RG,
                    ins=[snd_st.opt()], outs=[rcv_st.opt()])
                gst = PP.tile([1, D + 2], F32, tag="gst")
                nc.sync.dma_start(gst[:], rcv_st[:])
            else:
                gst = stat

            rS = SM.tile([1, 1], F32, tag="rS")
            nc.vector.reciprocal(rS[:], gst[:, 0:1])
            pooled = PP.tile([1, D], F32, tag="pooled")
            nc.vector.tensor_scalar_mul(pooled[:], gst[:, 2:D + 2], rS[:])
            nc.sync.dma_start(io["o_pool"][:], pooled[:])
            ao = PP.tile([1, NLOC], F32, tag="ao")
            nc.vector.tensor_scalar_mul(ao[:], attn_unr[:], rS[:])
            nc.sync.dma_start(io["o_attn"][:], ao[:])
            t5 = d768()
            nc.vector.tensor_tensor(out=t5[0:1, :], in0=pooled[:], in1=wcr[:],
                                    op=ALU.mult)
            t6 = SM.tile([1, 1], F32, tag="t6")
            nc.vector.tensor_reduce(out=t6[:], in_=t5[0:1, :], axis=AX.X,
                                    op=ALU.add)
            nc.vector.tensor_tensor(out=t6[:], in0=t6[:], in1=bcs[:], op=ALU.add)
            nc.sync.dma_start(io["o_log"][:], t6[:])


def _layernorm(nc, d768, SM, s2, g_row, b_row, out_ap, relu):
    """LN over free dim of s2 [128, D]; writes gamma/beta(+relu) to out_ap."""
    stats = SM.tile([128, 2, 6], F32, tag="lnstats")
    for c in range(2):
        nc.vector.bn_stats(stats[:, c, :], s2[:, c * 384:(c + 1) * 384])
    mv = SM.tile([128, 2], F32, tag="lnmv")
    nc.vector.bn_aggr(mv[:], stats[:])
    t3 = SM.tile([128, 1], F32, tag="lnt3")
    nc.vector.tensor_scalar_add(t3[:], mv[:, 1:2], LN_EPS)
    t4 = SM.tile([128, 1], F32, tag="lnt4")
    nc.scalar.activation(t4[:], t3[:], AF.Sqrt)
    rstd = SM.tile([128, 1], F32, tag="lnrstd")
    nc.vector.reciprocal(rstd[:], t4[:])
    s3 = d768()
    nc.vector.tensor_scalar(out=s3[:], in0=s2[:], scalar1=mv[:, 0:1],
                            scalar2=rstd[:], op0=ALU.subtract, op1=ALU.mult)
    s4 = d768()
    nc.gpsimd.tensor_tensor(out=s4[:], in0=s3[:], in1=g_row, op=ALU.mult)
    if relu:
        s5 = d768()
        nc.gpsimd.tensor_tensor(out=s5[:], in0=s4[:], in1=b_row, op=ALU.add)
        nc.scalar.activation(out_ap, s5[:], AF.Relu)
    else:
        nc.gpsimd.tensor_tensor(out=out_ap, in0=s4[:], in1=b_row, op=ALU.add)


_CACHE = {}


def _get_prog(pair=PAIR, debug=DEBUG):
    key = (pair, debug)
    if key not in _CACHE:
        _CACHE[key] = build(pair, debug)
    return _CACHE[key]


def make_in_maps(feats, W_gcn, ln_g, ln_b, outln_g, outln_b, W1, b1, W2, b2,
                 Wc, bc, pair=PAIR):
    f32 = np.float32
    feats = np.asarray(feats, f32)
    common = {
        "wgt": np.ascontiguousarray(np.asarray(W_gcn, f32).transpose(0, 2, 1)),
        "lng": np.ascontiguousarray(np.broadcast_to(
            np.asarray(ln_g, f32)[:, None, :], (2, 128, D))),
        "lnb": np.ascontiguousarray(np.broadcast_to(
            np.asarray(ln_b, f32)[:, None, :], (2, 128, D))),
        "og": np.ascontiguousarray(np.broadcast_to(
            np.asarray(outln_g, f32)[None, :], (128, D))),
        "ob": np.ascontiguousarray(np.broadcast_to(
            np.asarray(outln_b, f32)[None, :], (128, D))),
        "w1t": np.ascontiguousarray(np.asarray(W1, f32).T),
        "b1c": np.asarray(b1, f32).reshape(DA, 1),
        "w2c": np.ascontiguousarray(np.asarray(W2, f32).reshape(1, DA).T),
        "b2s": np.asarray(b2, f32).reshape(1, 1),
        "wcr": np.asarray(Wc, f32).reshape(1, D),
        "bcs": np.asarray(bc, f32).reshape(1, 1),
        "idf": np.eye(128, dtype=f32),
        "idb": np.eye(128, dtype=ml_dtypes.bfloat16),
        "aeye": (1.0 - np.eye(128)).astype(f32),
        "onesc": np.ones((128, 1), f32),
    }
    in_maps = []
    ncores = 8 if pair else 4
    for c in range(ncores):
        bag = c // 2 if pair else c
        h = c % 2 if pair else 0
        fb = feats[bag]
        if h == 1:
            fb = np.concatenate([fb[1024:], fb[:1024]], axis=0)
        d = dict(common)
        d["feats"] = np.ascontiguousarray(fb)
        in_maps.append(d)
    return in_maps


def run(inputs, pair=PAIR, debug=DEBUG, **spmd_kwargs):
    nc = _get_prog(pair, debug)
    in_maps = make_in_maps(
        inputs["feats"], inputs["W_gcn"], inputs["ln_g"], inputs["ln_b"],
        inputs["outln_g"], inputs["outln_b"], inputs["W1"], inputs["b1"],
        inputs["W2"], inputs["b2"], inputs["Wc"], inputs["bc"], pair=pair)
    ncores = 8 if pair else 4
    res = run_bass_kernel_spmd(nc, in_maps, core_ids=list(range(ncores)),
                               **spmd_kwargs)
    rs = res.results
    logits = np.zeros((B, 1), np.float32)
    pooled = np.zeros((B, D), np.float32)
    attn = np.zeros((B, N), np.float32)
    for bag in range(B):
        if pair:
            ev, od = rs[2 * bag], rs[2 * bag + 1]
            logits[bag, 0] = ev["o_log"][0, 0]
            pooled[bag] = ev["o_pool"][0]
            attn[bag, :1024] = ev["o_attn"][0]
            attn[bag, 1024:] = od["o_attn"][0]
        else:
            r = rs[bag]
            logits[bag, 0] = r["o_log"][0, 0]
            pooled[bag] = r["o_pool"][0]
            attn[bag] = r["o_attn"][0]
    return (logits, pooled, attn), res


def kernel(**inputs):
    out, _ = run(inputs)
    return out


if __name__ == "__main__":
    import reference
    ins = {k: np.asarray(v) for k, v in reference.setup_inputs().items()}
    out = kernel(**ins)
    print("logits:", out[0].ravel())


# revision 10
# speedup vs baseline: 4684.3054x; 1.3246x over previous
"""Trainium2 Bass kernel for nn_PatientMILFeatures (similarity-GCN + attention MIL pool).

Sharding: 4 bags x 2 cores/bag, row-split halves per bag. The odd core of each
pair gets half-rotated rows (the graph build / GCN / pooling are permutation
equivariant), so one SPMD program serves both pair members. Cross-core
exchanges use AllReduce(add) + subtract-recovery (no rank-dependent APs).

Numerics: similarity matmul in bf16 hi/lo (3 passes, ~1e-6 cos error) for
exact-enough top-8 selection via DVE max8 + match_replace (ties resolved to
lowest index, matching jax.lax.top_k). Adjacency stored bf16; GCN/attention
matmuls in f32r. Degree normalization folded into vector scalings.

Self-contained: hardcodes shapes; builds all constants on host.
"""
import sys
sys.path.insert(0, "/opt/trn_rl_repo")
import numpy as np
import ml_dtypes

import concourse.bacc as bacc
import concourse.tile as tile
import concourse.mybir as mybir
from concourse.bass_utils import run_bass_kernel_spmd

F32 = mybir.dt.float32
F32R = mybir.dt.float32r
BF16 = mybir.dt.bfloat16
AF = mybir.ActivationFunctionType
ALU = mybir.AluOpType
AX = mybir.AxisListType

B, N, D = 4, 2048, 768
KC = D // 128          # 6 contraction chunks
NB = N // 128          # 16 row/col blocks per bag
DA = 128               # attention dim
LN_EPS = 1e-5
RG = [[0, 1], [2, 3], [4, 5], [6, 7]]

PAIR = True
DEBUG = False


def build(pair=PAIR, debug=DEBUG):
    NLB = 8 if pair else 16
    NLOC = NLB * 128
    ncores = 8 if pair else 4
    nc = bacc.Bacc("TRN2", target_bir_lowering=False, debug=False,
                   num_devices=ncores)

    io = {}
    io["feats"] = nc.dram_tensor("feats", [N, D], F32, kind="ExternalInput").ap()
    io["wgt"] = nc.dram_tensor("wgt", [2, D, D], F32R, kind="ExternalInput").ap()
    io["lng"] = nc.dram_tensor("lng", [2, 128, D], F32, kind="ExternalInput").ap()
    io["lnb"] = nc.dram_tensor("lnb", [2, 128, D], F32, kind="ExternalInput").ap()
    io["og"] = nc.dram_tensor("og", [128, D], F32, kind="ExternalInput").ap()
    io["ob"] = nc.dram_tensor("ob", [128, D], F32, kind="ExternalInput").ap()
    io["w1t"] = nc.dram_tensor("w1t", [D, DA], F32R, kind="ExternalInput").ap()
    io["b1c"] = nc.dram_tensor("b1c", [DA, 1], F32, kind="ExternalInput").ap()
    io["w2c"] = nc.dram_tensor("w2c", [DA, 1], F32R, kind="ExternalInput").ap()
    io["b2s"] = nc.dram_tensor("b2s", [1, 1], F32, kind="ExternalInput").ap()
    io["wcr"] = nc.dram_tensor("wcr", [1, D], F32, kind="ExternalInput").ap()
    io["bcs"] = nc.dram_tensor("bcs", [1, 1], F32, kind="ExternalInput").ap()
    io["idf"] = nc.dram_tensor("idf", [128, 128], F32, kind="ExternalInput").ap()
    io["idb"] = nc.dram_tensor("idb", [128, 128], BF16, kind="ExternalInput").ap()
    io["aeye"] = nc.dram_tensor("aeye", [128, 128], F32, kind="ExternalInput").ap()
    io["onesc"] = nc.dram_tensor("onesc", [128, 1], F32R, kind="ExternalInput").ap()

    io["o_attn"] = nc.dram_tensor("o_attn", [1, NLOC], F32, kind="ExternalOutput").ap()
    io["o_pool"] = nc.dram_tensor("o_pool", [1, D], F32, kind="ExternalOutput").ap()
    io["o_log"] = nc.dram_tensor("o_log", [1, 1], F32, kind="ExternalOutput").ap()
    dbg = {}
    if debug:
        dbg["sim0"] = nc.dram_tensor("d_sim0", [128, N], F32, kind="ExternalOutput").ap()
        dbg["v80"] = nc.dram_tensor("d_v80", [128, 8], F32, kind="ExternalOutput").ap()
        dbg["T0"] = nc.dram_tensor("d_T0", [128, N], BF16, kind="ExternalOutput").ap()
        dbg["dis"] = nc.dram_tensor("d_dis", [128, NLB], F32, kind="ExternalOutput").ap()
        dbg["u0"] = nc.dram_tensor("d_u0", [128, D], BF16, kind="ExternalOutput").ap()
        dbg["x0"] = nc.dram_tensor("d_x0", [128, D], F32, kind="ExternalOutput").ap()
        dbg["xt0"] = nc.dram_tensor("d_xt0", [128, D], F32, kind="ExternalOutput").ap()
        dbg["lr"] = nc.dram_tensor("d_lr", [1, NLOC], F32, kind="ExternalOutput").ap()

    with tile.TileContext(nc) as tc:
        _body(nc, tc, pair, NLB, debug, dbg, io)
    nc.compile()
    return nc


def _body(nc, tc, pair, NLB, debug, dbg, io):
    NLOC = NLB * 128

    with tc.tile_pool(name="persist", bufs=1) as P, \
         tc.tile_pool(name="scratch", bufs=8) as SC, \
         tc.tile_pool(name="big", bufs=3) as BG, \
         tc.tile_pool(name="small", bufs=4) as SM, \
         tc.tile_pool(name="psum", bufs=3, space="PSUM") as PSF, \
         tc.tile_pool(name="psumb", bufs=2, space="PSUM") as PSB, \
         tc.tile_pool(name="dram", bufs=1, space="DRAM") as DR:

        def d768(dt=F32):
            return SC.tile([128, D], dt, tag="d768", name="d768")

        def d2048(rows=N, dt=F32):
            return BG.tile([128, rows], dt, tag="d2048", name="d2048")

        def pf(shape):
            return PSF.tile(shape, F32, tag="pf", name="pf")

        def pb():
            return PSB.tile([128, 1024], BF16, tag="pb", name="pb")

        # ---- persistent tiles (allocated up front; bottom of SBUF stack) ----
        idf = P.tile([128, 128], F32, tag="idf")
        idb = P.tile([128, 128], BF16, tag="idb")
        aeye = P.tile([128, 128], F32, tag="aeye")
        onesc = P.tile([128, 1], F32R, tag="onesc")
        lng = P.tile([128, 2, D], F32, tag="lng")
        lnb = P.tile([128, 2, D], F32, tag="lnb")
        og = P.tile([128, D], F32, tag="og")
        ob = P.tile([128, D], F32, tag="ob")
        w1t = P.tile([128, KC, DA], F32R, tag="w1t")
        b1c = P.tile([DA, 1], F32, tag="b1c")
        w2c = P.tile([DA, 1], F32R, tag="w2c")
        b2s = P.tile([1, 1], F32, tag="b2s")
        wcr = P.tile([1, D], F32, tag="wcr")
        bcs = P.tile([1, 1], F32, tag="bcs")
        x_my = P.tile([128, NLB, D], F32, tag="x_my")
        adjm = P.tile([128, NLB, N], BF16, tag="adjm")
        r_all = P.tile([128, NB], F32, tag="r_all")
        dis = P.tile([128, NLB], F32, tag="dis")
        wt = P.tile([128, KC, D], F32R, tag="wt")

        for name, tl in (("idf", idf), ("idb", idb), ("aeye", aeye),
                         ("onesc", onesc), ("lng", lng), ("lnb", lnb),
                         ("og", og), ("ob", ob), ("b1c", b1c), ("w2c", w2c),
                         ("b2s", b2s), ("wcr", wcr), ("bcs", bcs)):
            nc.sync.dma_start(tl[:], io[name][:])
        for k in range(KC):
            nc.sync.dma_start(w1t[:, k, :], io["w1t"][k * 128:(k + 1) * 128, :])
        feats = io["feats"]
        for i in range(NLB):
            nc.sync.dma_start(x_my[:, i, :], feats[i * 128:(i + 1) * 128, :])

        # ================= phase A: normalize + fnT hi/lo =================
        with tc.tile_pool(name="pFnT", bufs=1) as PF:
            fnT_hi = PF.tile([128, KC, N], BF16, tag="fnT_hi")
            fnT_lo = PF.tile([128, KC, N], BF16, tag="fnT_lo")
            for i in range(NB):
                if i < NLB:
                    xb = x_my[:, i, :]
                else:
                    xo = d768()
                    nc.sync.dma_start(
                        xo[:], feats[i * 128:(i + 1) * 128, :])
                    xb = xo[:]
                sq = d768()
                ss = SM.tile([128, 1], F32, tag="ss")
                nc.scalar.activation(sq[:], xb, AF.Square, accum_out=ss[:])
                nr = SM.tile([128, 1], F32, tag="nr")
                nc.scalar.activation(nr[:], ss[:], AF.Sqrt)
                nc.vector.tensor_scalar_max(nr[:], nr[:], 1e-8)
                nc.vector.reciprocal(r_all[:, i:i + 1], nr[:])
                fn = d768()
                nc.vector.tensor_scalar_mul(fn[:], xb, r_all[:, i:i + 1])
                pxt = pf([128, D])
                for k in range(KC):
                    nc.tensor.transpose(pxt[:, k * 128:(k + 1) * 128],
                                        fn[:, k * 128:(k + 1) * 128], idf[:])
                pxr = pxt[:].rearrange("p (k n) -> p k n", k=KC)
                hi_sl = fnT_hi[:, :, i * 128:(i + 1) * 128]
                nc.scalar.activation(hi_sl, pxr, AF.Copy)
                nc.vector.tensor_tensor(out=fnT_lo[:, :, i * 128:(i + 1) * 128],
                                        in0=pxr, in1=hi_sl, op=ALU.subtract)

            # ================= sim + top-8 per local row block =================
            for I in range(NLB):
                sim_sb = d2048()
                for ch in range(2):
                    ps = pf([128, 1024])
                    for c2 in range(2):
                        ops = [(pa, pb_, k)
                               for pa, pb_ in ((fnT_hi, fnT_hi), (fnT_hi, fnT_lo),
                                               (fnT_lo, fnT_hi))
                               for k in range(KC)]
                        for ix, (pa, pb_, k) in enumerate(ops):
                            col = ch * 1024 + c2 * 512
                            nc.tensor.matmul(
                                ps[:, c2 * 512:(c2 + 1) * 512],
                                pa[:, k, I * 128:(I + 1) * 128],
                                pb_[:, k, col:col + 512],
                                start=(ix == 0), stop=(ix == len(ops) - 1))
                    nc.scalar.activation(sim_sb[:, ch * 1024:(ch + 1) * 1024],
                                         ps[:], AF.Copy, bias=0.5, scale=0.5)
                nc.vector.tensor_tensor(out=sim_sb[:, I * 128:(I + 1) * 128],
                                        in0=sim_sb[:, I * 128:(I + 1) * 128],
                                        in1=aeye[:], op=ALU.mult)
                if debug and I == 0:
                    nc.sync.dma_start(dbg["sim0"][:], sim_sb[:])
                v8 = SM.tile([128, 8], F32, tag="v8")
                nc.vector.max(v8[:], sim_sb[:])
                if debug and I == 0:
                    nc.sync.dma_start(dbg["v80"][:], v8[:])
                repl = d2048()
                nc.vector.match_replace(repl[:], v8[:], sim_sb[:], 0.0)
                nc.vector.tensor_tensor(out=adjm[:, I, :], in0=sim_sb[:],
                                        in1=repl[:], op=ALU.subtract)

        # ================= symmetrize (+ pair exchange) =================
        with tc.tile_pool(name="phX", bufs=1) as PX:
            if pair:
                S_me = PX.tile([128, NLB, 1024], BF16, tag="S_me")
                snd_adj = DR.tile([NLB, 128, 1024], BF16, tag="snd_adj")
                rcv_adj = DR.tile([NLB, 128, 1024], BF16, tag="rcv_adj")
                for cb in range(NLB):
                    ptr = pb()
                    for rb in range(NLB):
                        nc.tensor.transpose(
                            ptr[:, rb * 128:(rb + 1) * 128],
                            adjm[:, rb, 1024 + cb * 128:1024 + (cb + 1) * 128],
                            idb[:])
                    nc.scalar.activation(S_me[:, cb, :], ptr[:], AF.Copy)
                    nc.sync.dma_start(snd_adj[cb, :, :], S_me[:, cb, :])
                nc.gpsimd.collective_compute(
                    "AllReduce", ALU.add, replica_groups=RG,
                    ins=[snd_adj.opt()], outs=[rcv_adj.opt()])

            atr = PX.tile([128, NLB, NLOC], BF16, tag="atr")
            for cb in range(NLB):
                for hf in range(NLB // 8):
                    ptr = pb()
                    for r8 in range(8):
                        rb = hf * 8 + r8
                        nc.tensor.transpose(ptr[:, r8 * 128:(r8 + 1) * 128],
                                            adjm[:, rb, cb * 128:(cb + 1) * 128],
                                            idb[:])
                    nc.vector.tensor_copy(
                        atr[:, cb, hf * 1024:(hf + 1) * 1024], ptr[:])
            for I in range(NLB):
                nc.vector.tensor_tensor(out=adjm[:, I, 0:NLOC],
                                        in0=adjm[:, I, 0:NLOC],
                                        in1=atr[:, I, :], op=ALU.add)
            if pair:
                for I in range(NLB):
                    t1 = d2048(1024, BF16)
                    nc.sync.dma_start(t1[:], rcv_adj[I, :, :])
                    part = d2048(1024)
                    nc.vector.tensor_tensor(out=part[:], in0=t1[:],
                                            in1=S_me[:, I, :], op=ALU.subtract)
                    nc.vector.tensor_tensor(out=adjm[:, I, 1024:2048],
                                            in0=adjm[:, I, 1024:2048],
                                            in1=part[:], op=ALU.add)
        if debug:
            nc.sync.dma_start(dbg["T0"][:], adjm[:, 0, :])

        # deg -> dis = (0.5*deg + 1)^-1/2
        for I in range(NLB):
            dg = SM.tile([128, 1], F32, tag="dg")
            nc.vector.tensor_reduce(out=dg[:], in_=adjm[:, I, :], axis=AX.X,
                                    op=ALU.add)
            nc.vector.tensor_scalar(out=dg[:], in0=dg[:], scalar1=0.5,
                                    scalar2=1.0, op0=ALU.mult, op1=ALU.add)
            sr = SM.tile([128, 1], F32, tag="sr")
            nc.scalar.activation(sr[:], dg[:], AF.Sqrt)
            nc.vector.reciprocal(dis[:, I:I + 1], sr[:])
        if debug:
            nc.sync.dma_start(dbg["dis"][:], dis[:])

        # ================= GCN layers =================
        with tc.tile_pool(name="phG", bufs=1) as PG:
            if pair:
                adjT = PG.tile([128, NLB, NLOC], BF16, tag="adjT")
                for mc in range(NLB):
                    ptr = pb()
                    for nb_ in range(NLB):
                        nc.tensor.transpose(
                            ptr[:, nb_ * 128:(nb_ + 1) * 128],
                            adjm[:, nb_, 1024 + mc * 128:1024 + (mc + 1) * 128],
                            idb[:])
                    nc.vector.tensor_copy(adjT[:, mc, :], ptr[:])
                snd_u = DR.tile([NLB, 128, D], BF16, tag="snd_u")
                rcv_u = DR.tile([NLB, 128, D], BF16, tag="rcv_u")
            nmc = NB if pair else NLB
            u_all = PG.tile([128, nmc, D], BF16, tag="u_all")
            y_bf = PG.tile([128, NLB, D], BF16, tag="y_bf")

            def lhsT_adj(mc, j):
                if mc < NLB:
                    return adjm[:, mc, j * 128:(j + 1) * 128]
                return adjT[:, mc - NLB, j * 128:(j + 1) * 128]

            for layer in range(2):
                for k in range(KC):
                    nc.sync.dma_start(
                        wt[:, k, :], io["wgt"][layer, k * 128:(k + 1) * 128, :])
                for i in range(NLB):
                    pxt = pf([128, D])
                    for k in range(KC):
                        nc.tensor.transpose(pxt[:, k * 128:(k + 1) * 128],
                                            x_my[:, i, k * 128:(k + 1) * 128],
                                            idf[:])
                    xtm = d768(F32R)
                    nc.scalar.activation(xtm[:], pxt[:], AF.Copy)
                    py = pf([128, D])
                    for k in range(KC):
                        for c0, cw in ((0, 512), (512, 256)):
                            nc.tensor.matmul(py[:, c0:c0 + cw],
                                             xtm[:, k * 128:(k + 1) * 128],
                                             wt[:, k, c0:c0 + cw],
                                             start=(k == 0), stop=(k == KC - 1))
                    nc.scalar.activation(y_bf[:, i, :], py[:], AF.Copy)
                for i in range(NLB):
                    nc.vector.tensor_scalar_mul(u_all[:, i, :], y_bf[:, i, :],
                                                dis[:, i:i + 1])
                    if pair:
                        nc.sync.dma_start(snd_u[i, :, :], u_all[:, i, :])
                if pair:
                    nc.gpsimd.collective_compute(
                        "AllReduce", ALU.add, replica_groups=RG,
                        ins=[snd_u.opt()], outs=[rcv_u.opt()])
                    for i in range(NLB):
                        t1 = d768(BF16)
                        t2 = d768(BF16)
                        nc.sync.dma_start(t1[:], rcv_u[i, :, :])
                        nc.sync.dma_start(t2[:], snd_u[i, :, :])
                        nc.vector.tensor_tensor(out=u_all[:, NLB + i, :],
                                                in0=t1[:], in1=t2[:],
                                                op=ALU.subtract)
                if debug and layer == 0:
                    nc.sync.dma_start(dbg["u0"][:], u_all[:, 0, :])
                for j in range(NLB):
                    pz = pf([128, D])
                    for mc in range(nmc):
                        for c0, cw in ((0, 512), (512, 256)):
                            nc.tensor.matmul(pz[:, c0:c0 + cw],
                                             lhsT_adj(mc, j),
                                             u_all[:, mc, c0:c0 + cw],
                                             start=(mc == 0), stop=(mc == nmc - 1))
                    s1 = d768()
                    nc.vector.scalar_tensor_tensor(
                        out=s1[:], in0=pz[:], scalar=0.5, in1=u_all[:, j, :],
                        op0=ALU.mult, op1=ALU.add)
                    s1b = d768()
                    nc.gpsimd.tensor_scalar_mul(s1b[:], s1[:], dis[:, j:j + 1])
                    s2 = d768()
                    nc.gpsimd.tensor_tensor(out=s2[:], in0=s1b[:],
                                            in1=x_my[:, j, :], op=ALU.add)
                    _layernorm(nc, d768, SM, s2, lng[:, layer, :],
                               lnb[:, layer, :], x_my[:, j, :], relu=True)
                if debug and layer == 0:
                    nc.sync.dma_start(dbg["x0"][:], x_my[:, 0, :])

        # ================= output LN =================
        for j in range(NLB):
            s2 = d768()
            nc.vector.tensor_copy(s2[:], x_my[:, j, :])
            _layernorm(nc, d768, SM, s2, og[:], ob[:], x_my[:, j, :], relu=False)
        if debug:
            nc.sync.dma_start(dbg["xt0"][:], x_my[:, 0, :])

        # ================= attention pool =================
        with tc.tile_pool(name="phP", bufs=1) as PP:
            xtT = PP.tile([128, KC, NLOC], F32R, tag="xtT")
            for j in range(NLB):
                pxt = pf([128, D])
                for k in range(KC):
                    nc.tensor.transpose(pxt[:, k * 128:(k + 1) * 128],
                                        x_my[:, j, k * 128:(k + 1) * 128], idf[:])
                nc.scalar.activation(xtT[:, :, j * 128:(j + 1) * 128],
                                     pxt[:].rearrange("p (k n) -> p k n", k=KC),
                                     AF.Copy)
            ph = pf([128, 1024])
            for c2 in range(NLOC // 512):
                for k in range(KC):
                    nc.tensor.matmul(ph[:, c2 * 512:(c2 + 1) * 512], w1t[:, k, :],
                                     xtT[:, k, c2 * 512:(c2 + 1) * 512],
                                     start=(k == 0), stop=(k == KC - 1))
            hT = PP.tile([128, NLOC], F32R, tag="hT")
            nc.scalar.activation(hT[:], ph[:], AF.Tanh, bias=b1c[:])

            plr = pf([1, 1024])
            for c2 in range(NLOC // 512):
                nc.tensor.matmul(plr[:, c2 * 512:(c2 + 1) * 512], w2c[:],
                                 hT[:, c2 * 512:(c2 + 1) * 512],
                                 start=True, stop=True)
            attn_unr = PP.tile([1, NLOC], F32, tag="attn_unr")
            S_my = SM.tile([1, 1], F32, tag="S_my")
            nc.scalar.activation(attn_unr[:], plr[:, 0:NLOC], AF.Exp,
                                 bias=b2s[:], accum_out=S_my[:])
            if debug:
                nc.sync.dma_start(dbg["lr"][:], attn_unr[:])

            plc = pf([128, NLB])
            for j in range(NLB):
                nc.tensor.matmul(plc[:, j:j + 1],
                                 hT[:, j * 128:(j + 1) * 128].bitcast(F32),
                                 w2c[:].bitcast(F32), start=True, stop=True)
            b2bc = SM.tile([128, 1], F32, tag="b2bc")
            nc.gpsimd.partition_broadcast(b2bc[:], b2s[:])
            wcol = SM.tile([128, NLB], F32, tag="wcol")
            nc.scalar.activation(wcol[:], plc[:], AF.Exp, bias=b2bc[:])

            pP = pf([1, D])
            for j in range(NLB):
                sc = d768(F32R)
                nc.vector.tensor_scalar_mul(sc[:], x_my[:, j, :],
                                            wcol[:, j:j + 1])
                for c0, cw in ((0, 512), (512, 256)):
                    nc.tensor.matmul(pP[:, c0:c0 + cw], onesc[:],
                                     sc[:, c0:c0 + cw],
                                     start=(j == 0), stop=(j == NLB - 1))

            stat = PP.tile([1, D + 2], F32, tag="stat")
            nc.vector.tensor_copy(stat[:, 0:1], S_my[:])
            nc.vector.memset(stat[:, 1:2], 0.0)
            nc.scalar.activation(stat[:, 2:D + 2], pP[:], AF.Copy)
            if pair:
                snd_st = DR.tile([1, D + 2], F32, tag="snd_st")
                rcv_st = DR.tile([1, D + 2], F32, tag="rcv_st")
                nc.sync.dma_start(snd_st[:], stat[:])
                nc.gpsimd.collective_compute(
                    "AllReduce", ALU.add, replica_groups=RG,
                    ins=[snd_st.opt()], outs=[rcv_st.opt()])
                gst = PP.tile([1, D + 2], F32, tag="gst")
                nc.sync.dma_start(gst[:], rcv_st[:])
            else:
                gst = stat

            rS = SM.tile([1, 1], F32, tag="rS")
            nc.vector.reciprocal(rS[:], gst[:, 0:1])
            pooled = PP.tile([1, D], F32, tag="pooled")
            nc.vector.tensor_scalar_mul(pooled[:], gst[:, 2:D + 2], rS[:])
            nc.sync.dma_start(io["o_pool"][:], pooled[:])
            ao = PP.tile([1, NLOC], F32, tag="ao")
            nc.vector.tensor_scalar_mul(ao[:], attn_unr[:], rS[:])
            nc.sync.dma_start(io["o_attn"][:], ao[:])
            t5 = d768()
            nc.vector.tensor_tensor(out=t5[0:1, :], in0=pooled[:], in1=wcr[:],
                                    op=ALU.mult)
            t6 = SM.tile([1, 1], F32, tag="t6")
            nc.vector.tensor_reduce(out=t6[:], in_=t5[0:1, :], axis=AX.X,
                                    op=ALU.add)
            nc.vector.tensor_tensor(out=t6[:], in0=t6[:], in1=bcs[:], op=ALU.add)
            nc.sync.dma_start(io["o_log"][:], t6[:])


def _layernorm(nc, d768, SM, s2, g_row, b_row, out_ap, relu):
    """LN over free dim of s2 [128, D]; writes gamma/beta(+relu) to out_ap."""
    stats = SM.tile([128, 2, 6], F32, tag="lnstats")
    for c in range(2):
        nc.vector.bn_stats(stats[:, c, :], s2[:, c * 384:(c + 1) * 384])
    mv = SM.tile([128, 2], F32, tag="lnmv")
    nc.vector.bn_aggr(mv[:], stats[:])
    t3 = SM.tile([128, 1], F32, tag="lnt3")
    nc.vector.tensor_scalar_add(t3[:], mv[:, 1:2], LN_EPS)
    t4 = SM.tile([128, 1], F32, tag="lnt4")
    nc.scalar.activation(t4[:], t3[:], AF.Sqrt)
    rstd = SM.tile([128, 1], F32, tag="lnrstd")
    nc.vector.reciprocal(rstd[:], t4[:])
    s3 = d768()
    nc.vector.tensor_scalar(out=s3[:], in0=s2[:], scalar1=mv[:, 0:1],
                            scalar2=rstd[:], op0=ALU.subtract, op1=ALU.mult)
    s4 = d768()
    nc.gpsimd.tensor_tensor(out=s4[:], in0=s3[:], in1=g_row, op=ALU.mult)
    if relu:
        s5 = d768()
        nc.gpsimd.tensor_tensor(out=s5[:], in0=s4[:], in1=b_row, op=ALU.add)
        nc.scalar.activation(out_ap, s5[:], AF.Relu)
    else:
        nc.gpsimd.tensor_tensor(out=out_ap, in0=s4[:], in1=b_row, op=ALU.add)


_CACHE = {}


def _get_prog(pair=PAIR, debug=DEBUG):
    key = (pair, debug)
    if key not in _CACHE:
        _CACHE[key] = build(pair, debug)
    return _CACHE[key]


def make_in_maps(feats, W_gcn, ln_g, ln_b, outln_g, outln_b, W1, b1, W2, b2,
                 Wc, bc, pair=PAIR):
    f32 = np.float32
    feats = np.asarray(feats, f32)
    common = {
        "wgt": np.ascontiguousarray(np.asarray(W_gcn, f32).transpose(0, 2, 1)),
        "lng": np.ascontiguousarray(np.broadcast_to(
            np.asarray(ln_g, f32)[:, None, :], (2, 128, D))),
        "lnb": np.ascontiguousarray(np.broadcast_to(
            np.asarray(ln_b, f32)[:, None, :], (2, 128, D))),
        "og": np.ascontiguousarray(np.broadcast_to(
            np.asarray(outln_g, f32)[None, :], (128, D))),
        "ob": np.ascontiguousarray(np.broadcast_to(
            np.asarray(outln_b, f32)[None, :], (128, D))),
        "w1t": np.ascontiguousarray(np.asarray(W1, f32).T),
        "b1c": np.asarray(b1, f32).reshape(DA, 1),
        "w2c": np.ascontiguousarray(np.asarray(W2, f32).reshape(1, DA).T),
        "b2s": np.asarray(b2, f32).reshape(1, 1),
        "wcr": np.asarray(Wc, f32).reshape(1, D),
        "bcs": np.asarray(bc, f32).reshape(1, 1),
        "idf": np.eye(128, dtype=f32),
        "idb": np.eye(128, dtype=ml_dtypes.bfloat16),
        "aeye": (1.0 - np.eye(128)).astype(f32),
        "onesc": np.ones((128, 1), f32),
    }
    in_maps = []
    ncores = 8 if pair else 4
    for c in range(ncores):
        bag = c // 2 if pair else c
        h = c % 2 if pair else 0
        fb = feats[bag]
        if h == 1:
            fb = np.concatenate([fb[1024:], fb[:1024]], axis=0)
        d = dict(common)
        d["feats"] = np.ascontiguousarray(fb)
        in_maps.append(d)
    return in_maps


def run(inputs, pair=PAIR, debug=DEBUG, **spmd_kwargs):
    nc = _get_prog(pair, debug)
    in_maps = make_in_maps(
        inputs["feats"], inputs["W_gcn"], inputs["ln_g"], inputs["ln_b"],
        inputs["outln_g"], inputs["outln_b"], inputs["W1"], inputs["b1"],
        inputs["W2"], inputs["b2"], inputs["Wc"], inputs["bc"], pair=pair)
    ncores = 8 if pair else 4
    res = run_bass_kernel_spmd(nc, in_maps, core_ids=list(range(ncores)),
                               **spmd_kwargs)
    rs = res.results
    logits = np.zeros((B, 1), np.float32)
    pooled = np.zeros((B, D), np.float32)
    attn = np.zeros((B, N), np.float32)
    for bag in range(B):
        if pair:
            ev, od = rs[2 * bag], rs[2 * bag + 1]
            logits[bag, 0] = ev["o_log"][0, 0]
            pooled[bag] = ev["o_pool"][0]
            attn[bag, :1024] = ev["o_attn"][0]
            attn[bag, 1024:] = od["o_attn"][0]
        else:
            r = rs[bag]
            logits[bag, 0] = r["o_log"][0, 0]
            pooled[bag] = r["o_pool"][0]
            attn[bag] = r["o_attn"][0]
    return (logits, pooled, attn), res


def kernel(**inputs):
    out, _ = run(inputs)
    return out


if __name__ == "__main__":
    import reference
    ins = {k: np.asarray(v) for k, v in reference.setup_inputs().items()}
    out = kernel(**ins)
    print("logits:", out[0].ravel())
